# revision 2
# baseline (speedup 1.0000x reference)
"""FlowNetC-style correlation (max_displacement=20, stride2=2, K=1) on 8 trn2 cores.

Math: out[b, ij, y, x] = (scale1*scale2/(96*out_scale)) *
        sum_c data1[b,c,y,x] * data2zp[b,c, y+dy, x+dx]
with ij = i*21 + j, dy = 2i-20, dx = 2j-20 and data2 zero-padded (pad cancels
against the output crop, so padding never materializes).

Strategy (per core = one batch element):
  - x is split by parity (dx is even so x and x+dx share parity): x = 2q+r;
    y likewise splits by parity yl (dy is even), y = 2*yh + yl.
  - Two y-rows (y0, y0+2) share one stationary operand (M=96 = 2x48 data1
    columns); the moving operand is the union of their 21-row data2 windows
    (22 rows), streamed once -- halving TensorE streaming vs per-row matmuls.
    PSUM tile [96, 22 slots x 48]: partition m = 48*g+q holds row y0+2g, slot
    s covers dy-index d0 = s-g.  The needed correlations are the 21 diagonals
    q' = q + dd of each [48,48] block.
  - Everything is bf16: inputs are cast host-side (error ~4e-3 rel, the
    harness gate is 2e-2), the matmul accumulates in fp32 PSUM, and the
    PSUM->SBUF copies (DVE/ACT alternating) downcast to bf16, halving both
    input and scratch-output HBM traffic vs fp32.
  - The 4 (yl, r) units of one row-pair (a "quad") share one stage tile and
    one batched ~670KB store DMA (out layout [yhp, m, yl, r, slots*48]),
    keeping store DMAs in the high-efficiency size regime.
  - Diagonals are gathered host-side with stride tricks (a per-partition-
    offset shear is not expressible on any engine AP); invalid (y,dy) slots
    are never written and read back as zeros (outputs are zero-initialized).
  - scale factor is folded into data1 on the host.
"""

import os

import ml_dtypes
import numpy as np

import concourse.bacc as bacc
import concourse.bass as bass
import concourse.mybir as mybir
import concourse.tile as tile
from concourse.bass_utils import run_bass_kernel_spmd

B, C, H, W = 8, 96, 64, 96
D = 21            # 21 displacements per axis (dy = 2*d0 - 20)
YH = H // 2       # 32  (y = 2*yh + yl)
Q = W // 2        # 48  (x = 2*q + r)
NSLOT = D + 1     # 22 dy-slots per row-pair (slot s -> d0 = s - g)
SLOTS_PER_BANK = 10   # 10 slots * 48 = 480 <= 512 fp32 per PSUM bank
BANK_F = 512
NBANKS = 3            # slots [0-9], [10-19], [20-21]
STAGE_F = NSLOT * Q   # 1056

COMPUTE_DT = os.environ.get("CORR_DT", "bf16")
NP_DT = {"bf16": ml_dtypes.bfloat16, "fp32": np.float32}

_NC = None
LAST_RESULT = None


def slot_range(yh):
    """Valid slots s for row-pair starting at yh (yyh = yh-10+s in [0,32))."""
    return max(0, 10 - yh), min(NSLOT - 1, 31 + 10 - yh)


def _chunks(slo, shi):
    out = []
    for k in range(NBANKS):
        a = max(slo, k * SLOTS_PER_BANK)
        b = min(shi, min((k + 1) * SLOTS_PER_BANK, NSLOT) - 1)
        if a <= b:
            out.append((k, a, b))
    return out


def build_nc(compute_dt=None):
    compute_dt = compute_dt or COMPUTE_DT
    cdt = mybir.dt.bfloat16 if compute_dt == "bf16" else mybir.dt.float32
    nc = bacc.Bacc("TRN2", target_bir_lowering=False, debug=False, num_devices=B)
    d1 = nc.dram_tensor("d1", [C, 2, 2, YH, Q], cdt, kind="ExternalInput")
    d2 = nc.dram_tensor("d2", [C, 2, 2, YH, Q], cdt, kind="ExternalInput")
    out = nc.dram_tensor(
        "out", [YH // 2, 2 * Q, 2, 2, STAGE_F], cdt, kind="ExternalOutput"
    )

    with tile.TileContext(nc) as tc:
        with (
            tc.tile_pool(name="inp", bufs=1) as inp,
            tc.tile_pool(name="psum", bufs=2, space=bass.MemorySpace.PSUM) as pp,
            tc.tile_pool(name="stage", bufs=3) as sp,
        ):
            s1 = inp.tile([C, 2, 2, YH, Q], cdt, tag="s1")
            s2 = inp.tile([C, 2, 2, YH, Q], cdt, tag="s2")
            # yl-sliced so the first quads' matmuls start early; s2 arrives
            # in yh-halves (rows 16+ are first needed by quad yhp=3)
            for yl in range(2):
                nc.sync.dma_start(s1[:, yl], d1[:, yl])
                nc.sync.dma_start(
                    s2[:, yl, :, 0 : YH // 2], d2[:, yl, :, 0 : YH // 2]
                )
            for yl in range(2):
                nc.sync.dma_start(
                    s2[:, yl, :, YH // 2 :], d2[:, yl, :, YH // 2 :]
                )

            unit = 0
            for yhp in range(YH // 2):
                yh = 2 * yhp
                slo, shi = slot_range(yh)
                ns = shi - slo + 1
                chunks = _chunks(slo, shi)

                st = sp.tile([2 * Q, 2, 2, STAGE_F], cdt, tag="st")
                for yl in range(2):
                    for r in range(2):
                        pt = pp.tile(
                            [2 * Q, NBANKS * BANK_F], mybir.dt.float32, tag="pt"
                        )
                        lhsT = s1[:, yl, r, yh : yh + 2, :]
                        for k, a, b in chunks:
                            rhs = s2[:, yl, r, yh - 10 + a : yh - 10 + b + 1, :]
                            po = k * BANK_F + (a - k * SLOTS_PER_BANK) * Q
                            n = (b - a + 1) * Q
                            nc.tensor.matmul(
                                pt[:, po : po + n], lhsT, rhs,
                                start=True, stop=True,
                            )

                        # split each unit's copies across DVE and ACT so the
                        # PSUM slot frees fast and the PE never idles on it
                        dst0 = 0
                        for ci, (k, a, b) in enumerate(chunks):
                            po = k * BANK_F + (a - k * SLOTS_PER_BANK) * Q
                            n = (b - a + 1) * Q
                            if (ci + unit) % 2 == 0:
                                nc.vector.tensor_copy(
                                    st[:, yl, r, dst0 : dst0 + n],
                                    pt[:, po : po + n],
                                )
                            else:
                                nc.scalar.copy(
                                    st[:, yl, r, dst0 : dst0 + n],
                                    pt[:, po : po + n],
                                )
                            dst0 += n
                        unit += 1

                nc.sync.dma_start(
                    out[yhp, :, :, :, slo * Q : (shi + 1) * Q],
                    st[:, :, :, 0 : ns * Q],
                )

    nc.compile()
    return nc


def _get_nc():
    global _NC
    if _NC is None:
        _NC = build_nc()
    return _NC


def _prep(x, dt):
    """[C, H, W] f32 -> [C, 2(yl), 2(r), YH, Q] contiguous, cast to dt."""
    return np.ascontiguousarray(
        x.reshape(C, YH, 2, Q, 2).transpose(0, 2, 4, 1, 3).astype(dt)
    )


def assemble(scratch, out_b):
    """Gather the 21 banded diagonals of each all-pairs tile into out_b.

    scratch: [YH//2, 96, 2, 2, STAGE_F] (zeros where never written).
    out_b:   [D*D, H, W] f32, pre-zeroed.
    """
    scratch = np.ascontiguousarray(scratch).astype(np.float32)
    outv = out_b.reshape(D, D, H, W)
    s_hp, s_m, s_yl, s_r, s_f = scratch.strides
    for yl in range(2):
        for r in range(2):
            for g in range(2):
                for dd in range(-10, 11):
                    q0 = max(0, -dd)
                    ln = Q - abs(dd)
                    base = scratch[
                        :, Q * g + q0 :, yl, r, Q * g + q0 + dd :
                    ]
                    view = np.lib.stride_tricks.as_strided(
                        base,
                        shape=(YH // 2, D, ln),
                        strides=(s_hp, Q * s_f, s_m + s_f),
                    )
                    outv[
                        :, dd + 10, yl + 2 * g :: 4,
                        r + 2 * q0 : r + 2 * (q0 + ln) : 2,
                    ] = view.swapaxes(0, 1)


def kernel(data1, data2, scale1, scale2, inter_scale, out_scale):
    data1 = np.asarray(data1, np.float32)
    data2 = np.asarray(data2, np.float32)
    factor = (
        float(np.asarray(scale1).reshape(-1)[0])
        * float(np.asarray(scale2).reshape(-1)[0])
        / (float(C) * float(np.asarray(out_scale).reshape(-1)[0]))
    )
    d1s = data1 * np.float32(factor)

    dt = NP_DT[COMPUTE_DT]
    in_maps = [
        {"d1": _prep(d1s[b], dt), "d2": _prep(data2[b], dt)} for b in range(B)
    ]
    res = run_bass_kernel_spmd(_get_nc(), in_maps, list(range(B)))
    global LAST_RESULT
    LAST_RESULT = res

    out = np.zeros((B, D * D, H, W), np.float32)
    for b in range(B):
        assemble(res.results[b]["out"], out[b])
    return out


# revision 4
# speedup vs baseline: 1.9690x; 1.9690x over previous
"""FlowNetC-style correlation (max_displacement=20, stride2=2, K=1) on 8 trn2 cores.

Math: out[b, ij, y, x] = (scale1*scale2/(96*out_scale)) *
        sum_c data1[b,c,y,x] * data2zp[b,c, y+dy, x+dx]
with ij = i*21 + j, dy = 2i-20, dx = 2j-20 and data2 zero-padded (pad cancels
against the output crop, so padding never materializes).

Strategy (per core = one batch element):
  - x is split by parity (dx is even so x and x+dx share parity): x = 2q+r;
    y likewise splits by parity yl (dy is even), y = 2*yh + yl.
  - Two y-rows (y0, y0+2) share one stationary operand (M=96 = 2x48 data1
    columns); the moving operand is the union of their 21-row data2 windows
    (22 rows), streamed once -- halving TensorE streaming vs per-row matmuls.
    PSUM tile [96, 22 slots x 48]: partition m = 48*g+q holds row y0+2g, slot
    s covers dy-index d0 = s-g.  The needed correlations are the 21 diagonals
    q' = q + dd of each [48,48] block.
  - Everything is bf16: inputs are cast host-side (error ~4e-3 rel, the
    harness gate is 2e-2), the matmul accumulates in fp32 PSUM, and the
    PSUM->SBUF copies (DVE/ACT alternating) downcast to bf16, halving both
    input and scratch-output HBM traffic vs fp32.
  - The 4 (yl, r) units of one row-pair (a "quad") share one stage tile and
    one batched ~670KB store DMA (out layout [yhp, m, yl, r, slots*48]),
    keeping store DMAs in the high-efficiency size regime.
  - Diagonals are gathered host-side with stride tricks (a per-partition-
    offset shear is not expressible on any engine AP); invalid (y,dy) slots
    are never written and read back as zeros (outputs are zero-initialized).
  - scale factor is folded into data1 on the host.
"""

import os

import ml_dtypes
import numpy as np

import concourse.bacc as bacc
import concourse.bass as bass
import concourse.mybir as mybir
import concourse.tile as tile
from concourse.bass_utils import run_bass_kernel_spmd

B, C, H, W = 8, 96, 64, 96
D = 21            # 21 displacements per axis (dy = 2*d0 - 20)
YH = H // 2       # 32  (y = 2*yh + yl)
Q = W // 2        # 48  (x = 2*q + r)
NSLOT = D + 1     # 22 dy-slots per row-pair (slot s -> d0 = s - g)
SLOTS_PER_BANK = 10   # 10 slots * 48 = 480 <= 512 fp32 per PSUM bank
BANK_F = 512
NBANKS = 3            # slots [0-9], [10-19], [20-21]
STAGE_F = NSLOT * Q   # 1056

COMPUTE_DT = os.environ.get("CORR_DT", "bf16")
NP_DT = {"bf16": ml_dtypes.bfloat16, "fp32": np.float32}

_NC = None
LAST_RESULT = None


def slot_range(yh):
    """Valid slots s for row-pair starting at yh (yyh = yh-10+s in [0,32))."""
    return max(0, 10 - yh), min(NSLOT - 1, 31 + 10 - yh)


def _chunks(slo, shi):
    out = []
    for k in range(NBANKS):
        a = max(slo, k * SLOTS_PER_BANK)
        b = min(shi, min((k + 1) * SLOTS_PER_BANK, NSLOT) - 1)
        if a <= b:
            out.append((k, a, b))
    return out


def build_nc(compute_dt=None):
    compute_dt = compute_dt or COMPUTE_DT
    cdt = mybir.dt.bfloat16 if compute_dt == "bf16" else mybir.dt.float32
    nc = bacc.Bacc("TRN2", target_bir_lowering=False, debug=False, num_devices=B)
    d1 = nc.dram_tensor("d1", [C, 2, 2, YH, Q], cdt, kind="ExternalInput")
    d2 = nc.dram_tensor("d2", [C, 2, 2, YH, Q], cdt, kind="ExternalInput")
    out = nc.dram_tensor(
        "out", [YH // 2, 2 * Q, 2, 2, STAGE_F], cdt, kind="ExternalOutput"
    )

    with tile.TileContext(nc) as tc:
        with (
            tc.tile_pool(name="inp", bufs=1) as inp,
            tc.tile_pool(name="psum", bufs=8, space=bass.MemorySpace.PSUM) as pp,
            tc.tile_pool(name="stage", bufs=3) as sp,
        ):
            s1 = inp.tile([C, 2, 2, YH, Q], cdt, tag="s1")
            s2 = inp.tile([C, 2, 2, YH, Q], cdt, tag="s2")
            # yl-sliced so the first quads' matmuls start early; s2 arrives
            # in yh-halves (rows 16+ are first needed by quad yhp=3)
            for yl in range(2):
                nc.sync.dma_start(s1[:, yl], d1[:, yl])
                nc.sync.dma_start(
                    s2[:, yl, :, 0 : YH // 2], d2[:, yl, :, 0 : YH // 2]
                )
            for yl in range(2):
                nc.sync.dma_start(
                    s2[:, yl, :, YH // 2 :], d2[:, yl, :, YH // 2 :]
                )

            # one PSUM bank per (unit, chunk): bufs=8 keeps ~2.7 units of
            # matmuls in flight ahead of the copies, so the PE streams
            # continuously (and thereby ramps to the 2.4 GHz p-state)
            big = 0  # alternates DVE/ACT for the 480-wide chunk copies
            for yhp in range(YH // 2):
                yh = 2 * yhp
                slo, shi = slot_range(yh)
                ns = shi - slo + 1
                chunks = _chunks(slo, shi)

                st = sp.tile([2 * Q, 2, 2, STAGE_F], cdt, tag="st")
                for yl in range(2):
                    for r in range(2):
                        lhsT = s1[:, yl, r, yh : yh + 2, :]
                        for k, a, b in chunks:
                            rhs = s2[:, yl, r, yh - 10 + a : yh - 10 + b + 1, :]
                            n = (b - a + 1) * Q
                            dst0 = (a - slo) * Q
                            pt = pp.tile(
                                [2 * Q, BANK_F], mybir.dt.float32, tag="pt"
                            )
                            nc.tensor.matmul(
                                pt[:, 0:n], lhsT, rhs, start=True, stop=True
                            )
                            # 96-wide chunk copies ride DVE (low fixed cost);
                            # 480-wide ones alternate DVE/ACT to balance
                            if n <= 2 * Q:
                                use_dve = True
                            else:
                                use_dve = big % 2 == 0
                                big += 1
                            if use_dve:
                                nc.vector.tensor_copy(
                                    st[:, yl, r, dst0 : dst0 + n], pt[:, 0:n]
                                )
                            else:
                                nc.scalar.copy(
                                    st[:, yl, r, dst0 : dst0 + n], pt[:, 0:n]
                                )

                nc.sync.dma_start(
                    out[yhp, :, :, :, slo * Q : (shi + 1) * Q],
                    st[:, :, :, 0 : ns * Q],
                )

    nc.compile()
    return nc


def _get_nc():
    global _NC
    if _NC is None:
        _NC = build_nc()
    return _NC


def _prep(x, dt):
    """[C, H, W] f32 -> [C, 2(yl), 2(r), YH, Q] contiguous, cast to dt."""
    return np.ascontiguousarray(
        x.reshape(C, YH, 2, Q, 2).transpose(0, 2, 4, 1, 3).astype(dt)
    )


def assemble(scratch, out_b):
    """Gather the 21 banded diagonals of each all-pairs tile into out_b.

    scratch: [YH//2, 96, 2, 2, STAGE_F] (zeros where never written).
    out_b:   [D*D, H, W] f32, pre-zeroed.
    """
    scratch = np.ascontiguousarray(scratch).astype(np.float32)
    outv = out_b.reshape(D, D, H, W)
    s_hp, s_m, s_yl, s_r, s_f = scratch.strides
    for yl in range(2):
        for r in range(2):
            for g in range(2):
                for dd in range(-10, 11):
                    q0 = max(0, -dd)
                    ln = Q - abs(dd)
                    base = scratch[
                        :, Q * g + q0 :, yl, r, Q * g + q0 + dd :
                    ]
                    view = np.lib.stride_tricks.as_strided(
                        base,
                        shape=(YH // 2, D, ln),
                        strides=(s_hp, Q * s_f, s_m + s_f),
                    )
                    outv[
                        :, dd + 10, yl + 2 * g :: 4,
                        r + 2 * q0 : r + 2 * (q0 + ln) : 2,
                    ] = view.swapaxes(0, 1)


def kernel(data1, data2, scale1, scale2, inter_scale, out_scale):
    data1 = np.asarray(data1, np.float32)
    data2 = np.asarray(data2, np.float32)
    factor = (
        float(np.asarray(scale1).reshape(-1)[0])
        * float(np.asarray(scale2).reshape(-1)[0])
        / (float(C) * float(np.asarray(out_scale).reshape(-1)[0]))
    )
    d1s = data1 * np.float32(factor)

    dt = NP_DT[COMPUTE_DT]
    in_maps = [
        {"d1": _prep(d1s[b], dt), "d2": _prep(data2[b], dt)} for b in range(B)
    ]
    res = run_bass_kernel_spmd(_get_nc(), in_maps, list(range(B)))
    global LAST_RESULT
    LAST_RESULT = res

    out = np.zeros((B, D * D, H, W), np.float32)
    for b in range(B):
        assemble(res.results[b]["out"], out[b])
    return out


# revision 12
# speedup vs baseline: 2.9083x; 1.4770x over previous
"""FlowNetC-style correlation (max_displacement=20, stride2=2, K=1) on 8 trn2 cores.

Math: out[b, ij, y, x] = (scale1*scale2/(96*out_scale)) *
        sum_c data1[b,c,y,x] * data2zp[b,c, y+dy, x+dx]
with ij = i*21 + j, dy = 2i-20, dx = 2j-20 and data2 zero-padded (pad cancels
against the output crop, so padding never materializes).

Strategy (per core = one batch element):
  - x splits by parity (dx is even): x = 2q+r, q in [0,48); y likewise:
    y = 2*yh + yl, yh in [0,32).  All four (yl, r) planes are independent.
  - Stationary operand = 8 consecutive yh-rows x 16 q-columns of data1
    (M=128, the full PE width).  This shape minimizes moving-operand
    streaming: the +-10 dy halo is amortized over 8 rows and the +-10 dd
    halo over 16 columns ((R+10)(w+20)/(R*w) is minimized at R=8, w=16
    given R*w=128).  Per row-group the moving operand is data2 rows
    [yh0-10, yh0+17] (clamped) x 3 overlapping q-windows (cols 26/36/26,
    clamped halos of the 16-wide stationary windows).
  - Everything is bf16 (harness gate 2e-2, this lands ~3.5e-3): inputs are
    cast host-side, matmul accumulates in fp32 PSUM, PSUM->SBUF copies
    (DVE/ACT, greedily balanced) downcast to bf16.
  - PSUM: one bank per matmul chunk (bufs=8) so the PE streams continuously.
  - Scratch DRAM layout [g4, m, yl, r, 2288]; store DMAs per (g4, yl)
    (~1MB each) alternate the two HWDGE queues (sync/scalar); input DMAs
    ride the gpsimd SWDGE queue so nothing serializes behind them.
  - The needed band elements (q'=q+dd diagonals) are gathered host-side via
    a precomputed flat index map; invalid (y+dy, x+dx) positions are zeros
    (never computed: out is pre-zeroed, gather skips them).
  - scale1*scale2/(96*out_scale) is folded into data1 on the host.
"""

import os

import ml_dtypes
import numpy as np

import concourse.bacc as bacc
import concourse.bass as bass
import concourse.mybir as mybir
import concourse.tile as tile
from concourse.bass_utils import run_bass_kernel_spmd

B, C, H, W = 8, 96, 64, 96
D = 21            # 21 displacements per axis
YH = H // 2       # 32  (y = 2*yh + yl)
Q = W // 2        # 48  (x = 2*q + r)
R8 = 8            # yh-rows per stationary group
W16 = 16          # q-columns per stationary window
NG = YH // R8     # 4 row-groups
NW = Q // W16     # 3 q-windows
WQ0 = [0, 6, 22]      # clamped moving-window start per w
WCOLS = [26, 36, 26]  # clamped moving-window width per w
WOFF = [0, 26, 62]    # per-slot scratch col offset prefix (units of n_s)
WB = [0, 26 * YH, 62 * YH]  # window base offsets in the flat d2w layout
D2FLAT = YH * 88      # 2816: flat (window, row, col) free dim of d2w
SUMW = 88             # sum of WCOLS
STAGE_F = 26 * SUMW   # 2288 (max n_s = 26)
BANK_F = 512

COMPUTE_DT = os.environ.get("CORR_DT", "bf16")
NP_DT = {"bf16": ml_dtypes.bfloat16, "fp32": np.float32}

_NC = None
_GATHER = None
LAST_RESULT = None


def grp(g4):
    """(yh0, y''0, n_s) for row-group g4."""
    yh0 = R8 * g4
    y0 = max(0, yh0 - 10)
    y1 = min(YH - 1, yh0 + R8 - 1 + 10)
    return yh0, y0, y1 - y0 + 1


def _chunks(n_s, w):
    """Split n_s slots into PSUM-bank chunks for window w."""
    spb = BANK_F // WCOLS[w]
    out = []
    a = 0
    while a < n_s:
        b = min(n_s, a + spb)
        out.append((a, b))
        a = b
    return out


def build_nc(compute_dt=None):
    compute_dt = compute_dt or COMPUTE_DT
    cdt = mybir.dt.bfloat16 if compute_dt == "bf16" else mybir.dt.float32
    nc = bacc.Bacc("TRN2", target_bir_lowering=False, debug=False, num_devices=B)
    # d1 pre-blocked per stationary tile; d2 pre-windowed (3 overlapping
    # q-windows materialized, window-major) — every matmul operand slice is
    # then a single contiguous free-dim run (a BIR Matmult requirement for
    # the stationary AP).
    d1 = nc.dram_tensor("d1", [C, 2, 2, NG, NW, R8, W16], cdt, kind="ExternalInput")
    d2 = nc.dram_tensor("d2", [C, 2, 2, D2FLAT], cdt, kind="ExternalInput")
    out = nc.dram_tensor(
        "out", [NG, 128, 2, 2, STAGE_F], cdt, kind="ExternalOutput"
    )

    with tile.TileContext(nc) as tc:
        with (
            tc.tile_pool(name="inp", bufs=1) as inp,
            tc.tile_pool(name="psum", bufs=8, space=bass.MemorySpace.PSUM) as pp,
            tc.tile_pool(name="stage", bufs=4) as sp,
        ):
            s1 = inp.tile([C, 2, 2, NG, NW, R8, W16], cdt, tag="s1")
            s2 = inp.tile([C, 2, 2, D2FLAT], cdt, tag="s2")
            # input DMAs ride the gpsimd SWDGE queue (separate from the two
            # HWDGE store queues); sliced so group 0 can start early
            for yl in range(2):
                nc.gpsimd.dma_start(s1[:, yl, :, 0:1], d1[:, yl, :, 0:1])
                for w in range(NW):
                    nc.gpsimd.dma_start(
                        s2[:, yl, :, WB[w] : WB[w] + 18 * WCOLS[w]],
                        d2[:, yl, :, WB[w] : WB[w] + 18 * WCOLS[w]],
                    )
            for yl in range(2):
                nc.gpsimd.dma_start(s1[:, yl, :, 1:], d1[:, yl, :, 1:])
                for w in range(NW):
                    a, b = WB[w] + 18 * WCOLS[w], WB[w] + YH * WCOLS[w]
                    nc.gpsimd.dma_start(
                        s2[:, yl, :, a:b], d2[:, yl, :, a:b]
                    )

            eng_load = [0, 0]  # greedy DVE/ACT balance (elements copied)
            ndma = 0
            for g4 in range(NG):
                yh0, y0, n_s = grp(g4)
                for yl in range(2):
                    st = sp.tile([128, 2, STAGE_F], cdt, tag="st")
                    for r in range(2):
                        for w in range(NW):
                            lhsT = s1[:, yl, r, g4, w]
                            wc = WCOLS[w]
                            for a, b in _chunks(n_s, w):
                                rhs = s2[
                                    :, yl, r,
                                    WB[w] + (y0 + a) * wc : WB[w] + (y0 + b) * wc,
                                ]
                                n = (b - a) * wc
                                dst0 = WOFF[w] * n_s + a * wc
                                pt = pp.tile(
                                    [128, BANK_F], mybir.dt.float32, tag="pt"
                                )
                                nc.tensor.matmul(
                                    pt[:, 0:n], lhsT, rhs, start=True, stop=True
                                )
                                # ACT has ~3.5x the fixed cost of DVE; weight
                                # it so greedy balancing accounts for that
                                cost = [n + 90, n + 310]
                                e = 0 if eng_load[0] + cost[0] <= eng_load[1] + cost[1] else 1
                                eng_load[e] += cost[e]
                                if e == 0:
                                    nc.vector.tensor_copy(
                                        st[:, r, dst0 : dst0 + n], pt[:, 0:n]
                                    )
                                else:
                                    nc.scalar.copy(
                                        st[:, r, dst0 : dst0 + n], pt[:, 0:n]
                                    )
                    dma_eng = nc.sync if ndma % 2 == 0 else nc.scalar
                    ndma += 1
                    dma_eng.dma_start(
                        out[g4, :, yl, :, 0 : n_s * SUMW],
                        st[:, :, 0 : n_s * SUMW],
                    )

    nc.compile()
    return nc


def _get_nc():
    global _NC
    if _NC is None:
        _NC = build_nc()
    return _NC


def _plane(x):
    """[C, H, W] -> [C, 2(yl), 2(r), YH, Q]."""
    return x.reshape(C, YH, 2, Q, 2).transpose(0, 2, 4, 1, 3)


def _prep1(x, dt):
    """data1 [C, H, W] -> blocked [C, 2, 2, NG, NW, R8, W16] contiguous."""
    p = _plane(x)  # [C, 2, 2, YH, Q]
    p = p.reshape(C, 2, 2, NG, R8, NW, W16).transpose(0, 1, 2, 3, 5, 4, 6)
    return np.ascontiguousarray(p.astype(dt))


def _prep2(x, dt):
    """data2 [C, H, W] -> windowed flat [C, 2, 2, D2FLAT] contiguous."""
    p = _plane(x)  # [C, 2, 2, YH, Q]
    blocks = [
        p[..., WQ0[w] : WQ0[w] + WCOLS[w]].reshape(C, 2, 2, YH * WCOLS[w])
        for w in range(NW)
    ]
    return np.ascontiguousarray(np.concatenate(blocks, axis=3).astype(dt))


def _build_gather():
    """Flat index map scratch -> out for the banded diagonals.

    scratch: [NG, 128, 2, 2, STAGE_F]; out: [D*D, H, W].
    """
    srcs, dsts = [], []
    ri, qq, d, dd = np.meshgrid(
        np.arange(R8), np.arange(W16), np.arange(-10, 11), np.arange(-10, 11),
        indexing="ij",
    )
    for g4 in range(NG):
        yh0, y0, n_s = grp(g4)
        for yl in range(2):
            for r in range(2):
                for w in range(NW):
                    y = yh0 + ri
                    ypp = y + d
                    q = W16 * w + qq
                    qp = q + dd
                    valid = (ypp >= 0) & (ypp < YH) & (qp >= 0) & (qp < Q)
                    s = ypp - y0
                    src = (
                        (((g4 * 128 + (W16 * ri + qq)) * 2 + yl) * 2 + r)
                        * STAGE_F
                        + WOFF[w] * n_s
                        + s * WCOLS[w]
                        + (qp - WQ0[w])
                    )
                    ch = (d + 10) * D + (dd + 10)
                    dst = (ch * H + 2 * y + yl) * W + 2 * q + r
                    srcs.append(src[valid])
                    dsts.append(dst[valid])
    return np.concatenate(srcs), np.concatenate(dsts)


def _gather():
    global _GATHER
    if _GATHER is None:
        _GATHER = _build_gather()
    return _GATHER


def assemble(scratch, out_b):
    """Gather banded diagonals of the all-pairs tiles into out_b (pre-zeroed)."""
    src, dst = _gather()
    sf = np.ascontiguousarray(scratch).astype(np.float32)
    out_b.reshape(-1)[dst] = sf.reshape(-1)[src]


def kernel(data1, data2, scale1, scale2, inter_scale, out_scale):
    data1 = np.asarray(data1, np.float32)
    data2 = np.asarray(data2, np.float32)
    factor = (
        float(np.asarray(scale1).reshape(-1)[0])
        * float(np.asarray(scale2).reshape(-1)[0])
        / (float(C) * float(np.asarray(out_scale).reshape(-1)[0]))
    )
    d1s = data1 * np.float32(factor)

    dt = NP_DT[COMPUTE_DT]
    in_maps = [
        {"d1": _prep1(d1s[b], dt), "d2": _prep2(data2[b], dt)}
        for b in range(B)
    ]
    res = run_bass_kernel_spmd(_get_nc(), in_maps, list(range(B)))
    global LAST_RESULT
    LAST_RESULT = res

    out = np.zeros((B, D * D, H, W), np.float32)
    for b in range(B):
        assemble(res.results[b]["out"], out[b])
    return out
